# revision 1
# baseline (speedup 1.0000x reference)
"""Approximate EMD loss (entropic Sinkhorn, 50 iters) on 8 TRN2 NeuronCores.

Pure data parallel: batch b -> core b. Each core runs a 2048x2048 Sinkhorn
entirely out of SBUF:
  - K = exp(-cost/eps) stored bf16 in BOTH orientations (K^T for the row
    update, K for the column update) so every matvec runs the TensorE fast
    path: vector stationary [128,1], matrix moving [128,512] (1 col/cycle).
  - The log-domain updates collapse to multiplicative form:
        e^u = C / (K e^v + eps_log),  C = 1/N + eps_log
    done by a fused ScalarE Reciprocal (PSUM -> SBUF row), then PE
    transposes reshape the [1,512] row chunks into [128,1] stationary
    columns for the next matvec.
  - Final EMD = e^u^T (K*cost) e^v with K*cost recomputed blockwise from
    K via cost = -eps*ln(max(K, tiny)) (exact 0 where K underflowed).
"""

import numpy as np

N = 2048
PB = 128                  # partition block
CHW = 512                 # psum chunk width (fp32 bank limit)
ITERS = 50
EPS_SINKHORN = 0.01
EPS_LOG = 1e-8
NCORES = 8


def _host_prep(X1, X2, n):
    """Per-batch host-side input prep (cheap O(N))."""
    X1 = np.ascontiguousarray(X1, dtype=np.float32)
    X2 = np.ascontiguousarray(X2, dtype=np.float32)
    A = (X1 * X1).sum(1).astype(np.float32)   # |x1_i|^2
    Bv = (X2 * X2).sum(1).astype(np.float32)  # |x2_j|^2
    ones = np.ones((1, n), np.float32)
    nb = n // PB
    # Layout A (K[i,j], i on partitions):  P' = x1e . x2e  with
    #   x1e=[x1,1], x2e=[x2,-B/2]  =>  K = exp(200*P' - 100*A_i)
    L1 = np.concatenate([X1.T, ones], 0)                  # [4, n] stationary
    R1 = np.concatenate([X2.T, (-Bv / 2)[None, :]], 0)    # [4, n] moving
    # Split each f32 operand into bf16 hi/mid/lo so the cost matmul can run
    # at bf16 speed (1 cyc/col instead of 4):  x = h + m + l + O(2^-25|x|).
    # dot(x,y) = hH + hM + mH + hL + lH + mM  (dropped terms < 1e-6).
    import ml_dtypes
    bf = ml_dtypes.bfloat16

    def split3(X):
        h = X.astype(bf)
        r = X - h.astype(np.float32)
        m = r.astype(bf)
        l = (r - m.astype(np.float32)).astype(bf)
        return h, m, l
    Lh, Lm, Ll = split3(L1)
    Rh, Rm, Rl = split3(R1)
    L1s = np.concatenate([Lh, Lh, Lm, Lh, Ll, Lm], 0)     # [24, n] bf16
    R1s = np.concatenate([Rh, Rm, Rh, Rl, Rh, Rm], 0)     # [24, n] bf16
    biasA = (-A / EPS_SINKHORN).astype(np.float32).reshape(nb, PB).T.copy()
    return {
        "L1": np.ascontiguousarray(L1s),
        "R1": np.ascontiguousarray(R1s),
        "biasA": np.ascontiguousarray(biasA),
    }


def build(nc, tc, ctx, aps, n=N, iters=ITERS):
    """Emit the single-core program. aps: dict name->dram AP."""
    import concourse.mybir as mybir

    f32 = mybir.dt.float32
    bf16 = mybir.dt.bfloat16
    AF = mybir.ActivationFunctionType
    ALU = mybir.AluOpType

    nb = n // PB            # number of 128-blocks
    nch = n // CHW          # number of 512-chunks
    tpc = CHW // PB         # transposes per chunk (4)
    C_MU = float(1.0 / n + EPS_LOG)
    ESCL = float(2.0 / EPS_SINKHORN)    # 200.0

    persist = ctx.enter_context(tc.tile_pool(name="persist", bufs=1))

    KA = persist.tile([PB, nb * n], bf16, tag="KA")   # [i_p, ib*n + j]
    KB = persist.tile([PB, nb * n], bf16, tag="KB")   # [j_p, jb*n + i]
    ev = persist.tile([PB, nb], bf16, tag="ev")       # e^v stationary cols
    eu = persist.tile([PB, nb], bf16, tag="eu")       # e^u stationary cols
    identB = persist.tile([PB, PB], bf16, tag="identB")
    ones_col = persist.tile([PB, 1], f32, tag="ones_col")
    tiny_col = persist.tile([PB, 1], f32, tag="tiny_col")
    biasA_sb = persist.tile([PB, nb], f32, tag="biasA")
    eu32 = persist.tile([PB, nb], f32, tag="eu32")
    persist_ps = ctx.enter_context(
        tc.tile_pool(name="persist_ps", bufs=1, space="PSUM"))
    wcol = persist_ps.tile([PB, 2 * nb], bf16, tag="wcol")

    from concourse.masks import make_identity

    nc.gpsimd.memset(ones_col[:, :], 1.0)
    nc.gpsimd.memset(tiny_col[:, :], 2e-38)
    nc.gpsimd.memset(ev[:, :], 1.0)   # e^{v_0} = 1
    make_identity(nc, identB[:, :])
    nc.sync.dma_start(out=biasA_sb[:, :], in_=aps["biasA"][:, :])

    # ---------------- setup: K_A via matmul+exp; K_B by transposing ----------
    with tc.tile_pool(name="sin", bufs=1) as sin, \
         tc.tile_pool(name="spsum", bufs=3, space="PSUM") as sp:
        L1 = sin.tile([24, n], bf16, tag="L1")
        R1 = sin.tile([24, n], bf16, tag="R1")
        for t, name in ((L1, "L1"), (R1, "R1")):
            nc.sync.dma_start(out=t[:, :], in_=aps[name][:, :])
        pending = None
        for ib in range(nb):
            for jc in range(nch):
                P = sp.tile([PB, CHW], f32, tag="P")
                nc.tensor.matmul(
                    P[:, :],
                    lhsT=L1[:, ib * PB:(ib + 1) * PB],
                    rhs=R1[:, jc * CHW:(jc + 1) * CHW],
                    start=True, stop=True,
                )
                nc.scalar.activation(
                    KA[:, ib * n + jc * CHW: ib * n + (jc + 1) * CHW],
                    P[:, :], AF.Exp,
                    bias=biasA_sb[:, ib:ib + 1], scale=ESCL,
                )
                if pending is not None:
                    pending()
                def mk_transpose(ib=ib, jc=jc):
                    # K_B[j, i] tiles by transposing the just-built K_A chunk
                    for q in range(tpc):
                        kbt = sp.tile([PB, PB], bf16, tag="kbt", name="kbt")
                        nc.tensor.transpose(
                            kbt[:, :],
                            KA[:, ib * n + jc * CHW + q * PB:
                               ib * n + jc * CHW + (q + 1) * PB],
                            identB[:, :],
                        )
                        nc.vector.tensor_copy(
                            KB[:, (jc * tpc + q) * n + ib * PB:
                               (jc * tpc + q) * n + (ib + 1) * PB],
                            kbt[:, :],
                        )
                pending = mk_transpose
        pending()

    # ---------------- Sinkhorn iterations ----------------
    rp = ctx.enter_context(tc.tile_pool(name="rp", bufs=5, space="PSUM"))
    tp = ctx.enter_context(tc.tile_pool(name="tp", bufs=2, space="PSUM"))
    rows = ctx.enter_context(tc.tile_pool(name="rows", bufs=4))

    def col(m):
        return m

    def half(mat, src, dst):
        """dst[:, :] (bf16 cols) = C / (matvec(mat, src) + eps)."""
        pending = None
        for c in range(nch):
            r = rp.tile([1, CHW], f32, tag="r", name="r")
            for jb in range(nb):
                nc.tensor.matmul(
                    r[0:1, :],
                    lhsT=src[:, jb:jb + 1],
                    rhs=mat[:, jb * n + c * CHW: jb * n + (c + 1) * CHW],
                    start=(jb == 0), stop=(jb == nb - 1),
                )
            if pending is not None:
                pending()
            def transform(c=c, r=r):
                # row = (r + eps)/C in bf16 (fused into the PSUM->SBUF copy);
                # bf16 rows make the PE transposes 1 cyc/row + fast-weight-load
                row = rows.tile([1, CHW], bf16, tag="brow", name="row")
                nc.scalar.activation(
                    row[0:1, :], r[0:1, :], AF.Copy,
                    bias=EPS_LOG / C_MU, scale=1.0 / C_MU,
                )
                # bf16 PSUM writes must be 4B-aligned -> pad columns 2x
                tcol = tp.tile([PB, 2 * tpc], bf16, tag="tcol", name="tcol")
                for t in range(tpc):
                    nc.tensor.transpose(
                        tcol[:, 2 * t:2 * t + 1],
                        row[0:1, t * PB:(t + 1) * PB],
                        identB[0:1, 0:1],
                    )
                tv = tcol.rearrange("p (t two) -> p t two", two=2)[:, :, 0]
                rec = rows.tile([PB, tpc], f32, tag="rec", name="rec")
                nc.vector.reciprocal(rec[:, :], tv)
                nc.vector.tensor_copy(dst[:, c * tpc:(c + 1) * tpc], rec[:, :])
            pending = transform
        pending()

    for _ in range(iters):
        half(KB, ev, eu)   # u-update: r_i = sum_j K[i,j] e^{v_j}
        half(KA, eu, ev)   # v-update: c_j = sum_i K[i,j] e^{u_i}

    # ---------------- final: emd = e^u^T (K*cost) e^v ----------------
    # (K*cost)^T = -eps * KB * ln(max(KB, tiny)); the -eps scale is folded
    # into the very last scalar copy.
    with tc.tile_pool(name="fin", bufs=4) as fin:
        nc.vector.tensor_copy(eu32[:, :], eu[:, :])
        ws = []
        for c in range(nch):
            ws.append(rp.tile([1, CHW], f32, tag="r", name=f"w{c}"))
        for jb in range(nb):
            kb_blk = KB[:, jb * n:(jb + 1) * n]
            # ln(K + tiny): the bias keeps ln finite where K underflowed to 0
            # (K * ln(...) is 0 there either way)
            lnk = fin.tile([PB, n], bf16, tag="lnk")
            nc.scalar.activation(lnk[:, :], kb_blk, AF.Ln,
                                 bias=tiny_col[:, 0:1], scale=1.0)
            mt = fin.tile([PB, n], bf16, tag="mt", bufs=10)  # ~(K*cost)^T/-eps
            nc.vector.tensor_mul(mt[:, :], kb_blk, lnk[:, :])
            for c in range(nch):
                nc.tensor.matmul(
                    ws[c][0:1, :],
                    lhsT=ev[:, jb:jb + 1],
                    rhs=mt[:, c * CHW:(c + 1) * CHW],
                    start=(jb == 0), stop=(jb == nb - 1),
                )
        for c in range(nch):
            wrow = rows.tile([1, CHW], bf16, tag="brow", name="wrow")
            nc.scalar.activation(wrow[0:1, :], ws[c][0:1, :], AF.Copy,
                                 bias=0.0, scale=1.0)
            for t in range(tpc):
                m = c * tpc + t
                nc.tensor.transpose(
                    wcol[:, 2 * m: 2 * m + 1],
                    wrow[0:1, t * PB:(t + 1) * PB],
                    identB[0:1, 0:1],
                )
        wv = wcol.rearrange("p (m two) -> p m two", two=2)[:, :, 0]
        prod = fin.tile([PB, nb], f32, tag="prod")
        dots = fin.tile([PB, 1], f32, tag="dots")
        nc.vector.tensor_mul(prod[:, :], wv, eu32[:, :])
        nc.vector.reduce_sum(dots[:, :], prod[:, :], axis=mybir.AxisListType.X)
        emd_ps = tp.tile([1, 1], f32, tag="tcol", name="emd_ps")
        nc.tensor.matmul(emd_ps[0:1, 0:1], lhsT=dots[:, 0:1],
                         rhs=ones_col[:, 0:1], start=True, stop=True)
        out_sb = fin.tile([1, 1], f32, tag="out_sb")
        nc.scalar.activation(out_sb[0:1, :], emd_ps[0:1, :], AF.Copy,
                             bias=0.0, scale=-EPS_SINKHORN)
        nc.sync.dma_start(out=aps["out"][:, :], in_=out_sb[0:1, :])


def _build_program(n=N, iters=ITERS, debug=False):
    from contextlib import ExitStack
    import concourse.mybir as mybir
    import concourse.tile as tile
    from concourse import bacc

    f32 = mybir.dt.float32
    nb = n // PB
    nc = bacc.Bacc(
        "TRN2",
        target_bir_lowering=False,
        debug=debug,
        enable_asserts=True,
        num_devices=NCORES,
    )
    aps = {}
    for name in ("L1", "R1"):
        aps[name] = nc.dram_tensor(
            name, [24, n], mybir.dt.bfloat16, kind="ExternalInput")[:, :]
    for name in ("biasA",):
        aps[name] = nc.dram_tensor(name, [PB, nb], f32, kind="ExternalInput")[:, :]
    aps["out"] = nc.dram_tensor("out", [1, 1], f32, kind="ExternalOutput")[:, :]
    with ExitStack() as ctx:
        tc = ctx.enter_context(tile.TileContext(nc))
        build(nc, tc, ctx, aps, n=n, iters=iters)
    nc.compile()
    return nc


_CACHE = {}
LAST_RESULT = None


def _install_ntff_hook_stub():
    """concourse's trace path imports antenv.axon_hooks unconditionally;
    some images lack it.  Provide a functional stub so trace=True (e.g. a
    BASS_TRACE env in the caller) can't crash the run."""
    import sys
    import types
    try:
        import antenv.axon_hooks  # noqa: F401
        return
    except ImportError:
        pass
    hook = None
    try:
        from trn_agent_boot.trn_boot import _ntff_profile_via_ctypes
        hook = _ntff_profile_via_ctypes("/opt/axon/libaxon_pjrt.so")
    except Exception:
        hook = None
    mod = types.ModuleType("antenv.axon_hooks")
    mod.get_axon_ntff_profile_hook = lambda: hook
    mod.set_axon_ntff_profile_hook = lambda h: None
    sys.modules["antenv.axon_hooks"] = mod


def kernel(x1, x2):
    global LAST_RESULT
    _install_ntff_hook_stub()
    from concourse.bass_utils import run_bass_kernel_spmd

    x1 = np.asarray(x1, dtype=np.float32)
    x2 = np.asarray(x2, dtype=np.float32)
    B = x1.shape[0]
    assert B == NCORES and x1.shape[1] == N

    if "nc" not in _CACHE:
        _CACHE["nc"] = _build_program()
    nc = _CACHE["nc"]

    in_maps = [_host_prep(x1[b], x2[b], N) for b in range(B)]
    res = run_bass_kernel_spmd(nc, in_maps, core_ids=list(range(NCORES)))
    LAST_RESULT = res
    out = np.array([res.results[b]["out"][0, 0] for b in range(B)],
                   dtype=np.float32)
    return out


if __name__ == "__main__":
    rng = np.random.default_rng(0)
    x1 = rng.standard_normal((NCORES, N, 3)).astype(np.float32)
    x2 = rng.standard_normal((NCORES, N, 3)).astype(np.float32)
    print(kernel(x1, x2))



# revision 16
# speedup vs baseline: 2.5889x; 2.5889x over previous
"""Approximate EMD loss (entropic Sinkhorn, 50 iters) on 8 TRN2 NeuronCores.

Pure data parallel: batch b -> core b. Each core runs a 2048x2048 Sinkhorn
entirely out of SBUF:
  - K = exp(-cost/eps) stored bf16 in BOTH orientations (K^T for the row
    update, K for the column update).
  - Each matvec runs 4-way column-tiled on the PE: four concurrent
    vector-stationary matmuls (tile_position=(0,32q)) stream four 512-col
    chunks of K at once -> ~4x the moving-operand bandwidth.
  - The four result rows land on psum partitions {0,32,64,96}. One ScalarE
    activation maps all four to SBUF ((r+eps)/C, bf16), then 4 "selector"
    matmuls (lhsT = 128-col row slice, rhs = 0/1 selector) transpose them
    into [128,4] columns each, and DVE reciprocals produce the next
    stationary vector e^u = C/(r+eps).
  - Final EMD = e^u^T (K*cost) e^v with K*cost = -eps*KB*ln(KB+tiny) built
    on ScalarE/DVE during the iterations; the closing dot-product divides
    by the last u-row directly (DVE scalar_tensor_tensor divide).
"""

import numpy as np

N = 2048
PB = 128                  # partition block
CHW = 512                 # psum chunk width (fp32 bank limit)
NB = N // PB              # 16
NCH = N // CHW            # 4
ITERS = 50
EPS_SINKHORN = 0.01
EPS_LOG = 1e-8
NCORES = 8
C_MU = float(1.0 / N + EPS_LOG)

# consumption order of j-blocks inside a half: blocks congruent to t mod 4
# become available together (selector matmul t), earliest first
PERM = [0, 4, 8, 12, 1, 5, 9, 13, 2, 6, 10, 14, 3, 7, 11, 15]


def _host_prep(X1, X2, n):
    """Per-batch host-side input prep (cheap O(N))."""
    X1 = np.ascontiguousarray(X1, dtype=np.float32)
    X2 = np.ascontiguousarray(X2, dtype=np.float32)
    A = (X1 * X1).sum(1).astype(np.float32)   # |x1_i|^2
    Bv = (X2 * X2).sum(1).astype(np.float32)  # |x2_j|^2
    ones = np.ones((1, n), np.float32)
    nb = n // PB
    # Layout A (K[i,j], i on partitions):  P' = x1e . x2e  with
    #   x1e=[x1,1], x2e=[x2,-B/2]  =>  K = exp(200*P' - 100*A_i)
    L1 = np.concatenate([X1.T, ones], 0)                  # [4, n] stationary
    R1 = np.concatenate([X2.T, (-Bv / 2)[None, :]], 0)    # [4, n] moving
    # Split each f32 operand into bf16 hi/mid/lo so the cost matmul can run
    # at bf16 speed:  dot(x,y) = hH + hM + mH + hL + lH + mM.
    import ml_dtypes
    bf = ml_dtypes.bfloat16

    def split3(X):
        h = X.astype(bf)
        r = X - h.astype(np.float32)
        m = r.astype(bf)
        l = (r - m.astype(np.float32)).astype(bf)
        return h, m, l
    Lh, Lm, Ll = split3(L1)
    Rh, Rm, Rl = split3(R1)
    L1s = np.concatenate([Lh, Lh, Lm, Lh, Ll, Lm], 0)     # [24, n] bf16
    R1s = np.concatenate([Rh, Rm, Rh, Rl, Rh, Rm], 0)     # [24, n] bf16
    biasA = (-A / EPS_SINKHORN).astype(np.float32).reshape(nb, PB).T.copy()
    return {
        "L1": np.ascontiguousarray(L1s),
        "R1": np.ascontiguousarray(R1s),
        "biasA": np.ascontiguousarray(biasA),
    }


def build(nc, tc, ctx, aps, n=N, iters=ITERS):
    """Emit the single-core program. aps: dict name->dram AP."""
    import concourse.mybir as mybir

    f32 = mybir.dt.float32
    bf16 = mybir.dt.bfloat16
    AF = mybir.ActivationFunctionType
    ALU = mybir.AluOpType

    nb = n // PB            # 16
    nch = n // CHW          # 4
    tpc = CHW // PB         # 4
    ESCL = float(2.0 / EPS_SINKHORN)    # 200.0

    persist = ctx.enter_context(tc.tile_pool(name="persist", bufs=1))

    KA = persist.tile([PB, nb * n], bf16, tag="KA")   # [i_p, ib*n + j]
    KB = persist.tile([PB, nb * n], bf16, tag="KB")   # [j_p, jb*n + i]
    ev = persist.tile([PB, nb], bf16, tag="ev")       # e^v stationary cols
    eu = persist.tile([PB, nb], bf16, tag="eu")       # e^u stationary cols
    identB = persist.tile([PB, PB], bf16, tag="identB")
    tiny_col = persist.tile([PB, 1], f32, tag="tiny_col")
    biasA_sb = persist.tile([PB, nb], f32, tag="biasA")
    selS = persist.tile([97, tpc], bf16, tag="selS")    # selector 0/1
    ones_col = persist.tile([PB, 1], f32, tag="ones_col")

    from concourse.masks import make_identity

    nc.gpsimd.memset(tiny_col[:, :], 2e-38)
    nc.gpsimd.memset(ev[:, :], 1.0)   # e^{v_0} = 1
    nc.gpsimd.memset(selS[:, :], 0.0)
    nc.gpsimd.memset(ones_col[:, :], 1.0)
    for c in range(4):
        nc.gpsimd.memset(selS[32 * c:32 * c + 1, c:c + 1], 1.0)
    make_identity(nc, identB[:, :])
    nc.sync.dma_start(out=biasA_sb[:, :], in_=aps["biasA"][:, :])

    # ---------------- setup: K_A via matmul+exp; K_B by transposing ----------
    with tc.tile_pool(name="sin", bufs=1) as sin, \
         tc.tile_pool(name="spsum", bufs=3, space="PSUM") as sp:
        L1 = sin.tile([24, n], bf16, tag="L1")
        R1 = sin.tile([24, n], bf16, tag="R1")
        for t, name in ((L1, "L1"), (R1, "R1")):
            nc.sync.dma_start(out=t[:, :], in_=aps[name][:, :])
        pending = None
        for ib in range(nb):
            for jc in range(nch):
                P = sp.tile([PB, CHW], f32, tag="P")
                nc.tensor.matmul(
                    P[:, :],
                    lhsT=L1[:, ib * PB:(ib + 1) * PB],
                    rhs=R1[:, jc * CHW:(jc + 1) * CHW],
                    start=True, stop=True,
                )
                nc.scalar.activation(
                    KA[:, ib * n + jc * CHW: ib * n + (jc + 1) * CHW],
                    P[:, :], AF.Exp,
                    bias=biasA_sb[:, ib:ib + 1], scale=ESCL,
                )
                if pending is not None:
                    pending()
                def mk_transpose(ib=ib, jc=jc):
                    # K_B[j, i] tiles by transposing the just-built K_A chunk
                    for q in range(tpc):
                        kbt = sp.tile([PB, PB], bf16, tag="kbt", name="kbt")
                        nc.tensor.transpose(
                            kbt[:, :],
                            KA[:, ib * n + jc * CHW + q * PB:
                               ib * n + jc * CHW + (q + 1) * PB],
                            identB[:, :],
                        )
                        nc.vector.tensor_copy(
                            KB[:, (jc * tpc + q) * n + ib * PB:
                               (jc * tpc + q) * n + (ib + 1) * PB],
                            kbt[:, :],
                        )
                pending = mk_transpose
        pending()

    # ---------------- Sinkhorn iterations (4-way column-tiled) ----------------
    rp = ctx.enter_context(tc.tile_pool(name="rp", bufs=2, space="PSUM"))
    tp = ctx.enter_context(tc.tile_pool(name="tp", bufs=4, space="PSUM"))
    rows = ctx.enter_context(tc.tile_pool(name="rows", bufs=2))

    # initialize all 128 partitions of both R psum banks so the [97,512]
    # ScalarE read below never sees uninitialized psum
    for _ in range(2):
        Rinit = rp.tile([PB, CHW], f32, tag="R", name="Rinit")
        nc.tensor.matmul(Rinit[:, :], lhsT=identB[:, :], rhs=KA[:, 0:CHW],
                         start=True, stop=True)

    def half(mat, src, dst, prev_transform):
        """dst cols = C/(matvec(mat, src) + eps); returns transform closure."""
        R = rp.tile([PB, CHW], f32, tag="R", name="R")
        for g in range(nb):
            jb = PERM[g]
            if prev_transform is not None and g == 0:
                for t in range(tpc):
                    prev_transform(t)
            for q in range(4):
                nc.tensor.matmul(
                    R[32 * q:32 * q + 1, :],
                    lhsT=src[:, jb:jb + 1],
                    rhs=mat[:, jb * n + q * CHW: jb * n + (q + 1) * CHW],
                    start=(g == 0), stop=(g == nb - 1),
                    tile_position=(0, 32 * q),
                )
        srow = rows.tile([97, CHW], bf16, tag="srow", name="srow")

        def transform(t, R=R, srow=srow):
            if t == 0:
                nc.scalar.activation(
                    srow[:, :], R[0:97, :], AF.Copy,
                    bias=EPS_LOG / C_MU, scale=1.0 / C_MU)
            selps = tp.tile([PB, tpc], f32, tag="selps", name=f"selps{t}")
            nc.tensor.matmul(
                selps[:, :],
                lhsT=srow[:, PB * t:PB * (t + 1)],
                rhs=selS[:, :],
                start=True, stop=True,
            )
            # selps col c holds block (4c + t)
            dv = dst.rearrange("p (c t) -> p t c", t=tpc)[:, t, :]
            with nc.allow_low_precision(reason="ev/eu are stored bf16 anyway"):
                nc.vector.reciprocal(dv, selps[:, :])

        return transform

    # mt_jb = KB_jb * ln(KB_jb + tiny) = (K*cost)^T / -eps, built on
    # ScalarE/DVE interleaved with the iterations (they are ~85% idle).
    fin = ctx.enter_context(tc.tile_pool(name="fin", bufs=1))
    mts = []

    def build_mt(jb):
        kb_blk = KB[:, jb * n:(jb + 1) * n]
        lnk = fin.tile([PB, n], bf16, tag="lnk", bufs=2, name=f"lnk{jb}")
        nc.scalar.activation(lnk[:, :], kb_blk, AF.Ln,
                             bias=tiny_col[:, 0:1], scale=1.0)
        mt = fin.tile([PB, n], bf16, tag="mt", bufs=nb, name=f"mt{jb}")
        nc.vector.tensor_mul(mt[:, :], kb_blk, lnk[:, :])
        mts.append(mt)

    pend = None
    for it in range(iters):
        pend = half(KB, ev, eu, pend)
        if 4 <= it < 4 + nb:
            build_mt(it - 4)
        pend = half(KA, eu, ev, pend)

    # ---------------- final: emd = e^u^T (K*cost) e^v ----------------
    import os
    if os.environ.get("KCUT"):
        for t in range(tpc):
            pend(t)
        out_dbg = fin.tile([1, 1], f32, tag="out_dbg")
        nc.vector.tensor_copy(out_dbg[0:1, 0:1], ev[0:1, 0:1])
        nc.sync.dma_start(out=aps["out"][:, :], in_=out_dbg[0:1, :])
        return

    # w rows: col-tiled matvec of mt with ev stationary (consumes pend)
    W = rp.tile([PB, CHW], f32, tag="R", name="W")
    for g in range(nb):
        jb = PERM[g]
        if g == 0:
            for t in range(tpc):
                pend(t)
        for q in range(4):
            nc.tensor.matmul(
                W[32 * q:32 * q + 1, :],
                lhsT=ev[:, jb:jb + 1],
                rhs=mts[jb][:, q * CHW:(q + 1) * CHW],
                start=(g == 0), stop=(g == nb - 1),
                tile_position=(0, 32 * q),
            )

    # dot: emd = -eps * sum_i w_i * e^u_i. Transpose W's rows into columns
    # via selector matmuls, multiply by the eu columns, reduce.
    wsrow = fin.tile([97, CHW], bf16, tag="wsrow")
    nc.scalar.activation(wsrow[:, :], W[0:97, :], AF.Copy, bias=0.0, scale=1.0)
    if os.environ.get("KCUT2"):
        out_dbg = fin.tile([1, 1], f32, tag="out_dbg")
        nc.vector.tensor_copy(out_dbg[0:1, 0:1], wsrow[0:1, 0:1])
        nc.sync.dma_start(out=aps["out"][:, :], in_=out_dbg[0:1, :])
        return
    accums = [fin.tile([PB, 1], f32, tag="accum", bufs=2, name=f"acc{t}")
              for t in range(tpc)]
    for t in range(tpc):
        wps = tp.tile([PB, tpc], f32, tag="selps", name=f"wps{t}")
        nc.tensor.matmul(
            wps[:, :], lhsT=wsrow[:, PB * t:PB * (t + 1)], rhs=selS[:, :],
            start=True, stop=True)
        euv = eu.rearrange("p (c t) -> p t c", t=tpc)[:, t, :]
        prod = fin.tile([PB, tpc], f32, tag="prod", bufs=2, name=f"prod{t}")
        nc.vector.tensor_tensor_reduce(
            prod[:, :], wps[:, :], euv, 1.0,
            0.0 if t == 0 else accums[t - 1][:, 0:1],
            op0=ALU.mult, op1=ALU.add,
            accum_out=accums[t][:, 0:1],
        )
        if os.environ.get("KCUT3"):
            break
    if os.environ.get("KCUT3"):
        out_dbg = fin.tile([1, 1], f32, tag="out_dbg")
        nc.vector.tensor_copy(out_dbg[0:1, 0:1], accums[0][0:1, 0:1])
        nc.sync.dma_start(out=aps["out"][:, :], in_=out_dbg[0:1, :])
        return
    emd_ps = tp.tile([1, 1], f32, tag="selps", name="emd_ps")
    nc.tensor.matmul(emd_ps[0:1, 0:1], lhsT=accums[-1][:, 0:1],
                     rhs=ones_col[:, 0:1], start=True, stop=True)
    out_sb = fin.tile([1, 1], f32, tag="out_sb")
    nc.scalar.activation(out_sb[0:1, :], emd_ps[0:1, :], AF.Copy,
                         bias=0.0, scale=-EPS_SINKHORN)
    nc.sync.dma_start(out=aps["out"][:, :], in_=out_sb[0:1, :])


def _build_program(n=N, iters=ITERS, debug=False):
    from contextlib import ExitStack
    import concourse.mybir as mybir
    import concourse.tile as tile
    from concourse import bacc

    f32 = mybir.dt.float32
    nb = n // PB
    nc = bacc.Bacc(
        "TRN2",
        target_bir_lowering=False,
        debug=debug,
        enable_asserts=True,
        num_devices=NCORES,
    )
    aps = {}
    for name in ("L1", "R1"):
        aps[name] = nc.dram_tensor(
            name, [24, n], mybir.dt.bfloat16, kind="ExternalInput")[:, :]
    for name in ("biasA",):
        aps[name] = nc.dram_tensor(name, [PB, nb], f32, kind="ExternalInput")[:, :]
    aps["out"] = nc.dram_tensor("out", [1, 1], f32, kind="ExternalOutput")[:, :]
    with ExitStack() as ctx:
        tc = ctx.enter_context(tile.TileContext(nc))
        build(nc, tc, ctx, aps, n=n, iters=iters)
    nc.compile()
    return nc


_CACHE = {}
LAST_RESULT = None


def _install_ntff_hook_stub():
    """concourse's trace path imports antenv.axon_hooks unconditionally;
    some images lack it.  Provide a functional stub so trace=True (e.g. a
    BASS_TRACE env in the caller) can't crash the run."""
    import sys
    import types
    try:
        import antenv.axon_hooks  # noqa: F401
        return
    except ImportError:
        pass
    hook = None
    try:
        from trn_agent_boot.trn_boot import _ntff_profile_via_ctypes
        hook = _ntff_profile_via_ctypes("/opt/axon/libaxon_pjrt.so")
    except Exception:
        hook = None
    mod = types.ModuleType("antenv.axon_hooks")
    mod.get_axon_ntff_profile_hook = lambda: hook
    mod.set_axon_ntff_profile_hook = lambda h: None
    sys.modules["antenv.axon_hooks"] = mod


def kernel(x1, x2):
    global LAST_RESULT
    _install_ntff_hook_stub()
    from concourse.bass_utils import run_bass_kernel_spmd

    x1 = np.asarray(x1, dtype=np.float32)
    x2 = np.asarray(x2, dtype=np.float32)
    B = x1.shape[0]
    assert B == NCORES and x1.shape[1] == N

    if "nc" not in _CACHE:
        _CACHE["nc"] = _build_program()
    nc = _CACHE["nc"]

    in_maps = [_host_prep(x1[b], x2[b], N) for b in range(B)]
    res = run_bass_kernel_spmd(nc, in_maps, core_ids=list(range(NCORES)))
    LAST_RESULT = res
    out = np.array([res.results[b]["out"][0, 0] for b in range(B)],
                   dtype=np.float32)
    return out


if __name__ == "__main__":
    rng = np.random.default_rng(0)
    x1 = rng.standard_normal((NCORES, N, 3)).astype(np.float32)
    x2 = rng.standard_normal((NCORES, N, 3)).astype(np.float32)
    print(kernel(x1, x2))


# revision 17
# speedup vs baseline: 3.0980x; 1.1966x over previous
"""Approximate EMD loss (entropic Sinkhorn, 50 iters) on 8 TRN2 NeuronCores.

Pure data parallel: batch b -> core b. Each core runs a 2048x2048 Sinkhorn
entirely out of SBUF:
  - K = exp(-cost/eps) stored bf16 in BOTH orientations (K^T for the row
    update, K for the column update).
  - Each matvec runs 4-way column-tiled on the PE: four concurrent
    vector-stationary matmuls (tile_position=(0,32q)) stream four 512-col
    chunks of K at once -> ~4x the moving-operand bandwidth.
  - The four result rows land on psum partitions {0,32,64,96}. One ScalarE
    activation maps all four to SBUF ((r+eps)/C, bf16), then 4 "selector"
    matmuls (lhsT = 128-col row slice, rhs = 0/1 selector) transpose them
    into [128,4] columns each, and DVE reciprocals produce the next
    stationary vector e^u = C/(r+eps).
  - Final EMD = e^u^T (K*cost) e^v with K*cost = -eps*KB*ln(KB+tiny) built
    on ScalarE/DVE during the iterations; the closing dot-product divides
    by the last u-row directly (DVE scalar_tensor_tensor divide).
"""

import numpy as np

N = 2048
PB = 128                  # partition block
CHW = 512                 # psum chunk width (fp32 bank limit)
NB = N // PB              # 16
NCH = N // CHW            # 4
ITERS = 50
EPS_SINKHORN = 0.01
EPS_LOG = 1e-8
NCORES = 8
C_MU = float(1.0 / N + EPS_LOG)

# consumption order of j-blocks inside a half: blocks congruent to t mod 4
# become available together (selector matmul t), earliest first
PERM = [0, 4, 8, 12, 1, 5, 9, 13, 2, 6, 10, 14, 3, 7, 11, 15]


def _host_prep(X1, X2, n):
    """Per-batch host-side input prep (cheap O(N))."""
    X1 = np.ascontiguousarray(X1, dtype=np.float32)
    X2 = np.ascontiguousarray(X2, dtype=np.float32)
    A = (X1 * X1).sum(1).astype(np.float32)   # |x1_i|^2
    Bv = (X2 * X2).sum(1).astype(np.float32)  # |x2_j|^2
    ones = np.ones((1, n), np.float32)
    nb = n // PB
    # Layout A (K[i,j], i on partitions):  P' = x1e . x2e  with
    #   x1e=[x1,1], x2e=[x2,-B/2]  =>  K = exp(200*P' - 100*A_i)
    L1 = np.concatenate([X1.T, ones], 0)                  # [4, n] stationary
    R1 = np.concatenate([X2.T, (-Bv / 2)[None, :]], 0)    # [4, n] moving
    # Split each f32 operand into bf16 hi/mid/lo so the cost matmul can run
    # at bf16 speed:  dot(x,y) = hH + hM + mH + hL + lH + mM.
    import ml_dtypes
    bf = ml_dtypes.bfloat16

    def split3(X):
        h = X.astype(bf)
        r = X - h.astype(np.float32)
        m = r.astype(bf)
        l = (r - m.astype(np.float32)).astype(bf)
        return h, m, l
    Lh, Lm, Ll = split3(L1)
    Rh, Rm, Rl = split3(R1)
    L1s = np.concatenate([Lh, Lh, Lm, Lh, Ll, Lm], 0)     # [24, n] bf16
    R1s = np.concatenate([Rh, Rm, Rh, Rl, Rh, Rm], 0)     # [24, n] bf16
    biasA = (-A / EPS_SINKHORN).astype(np.float32).reshape(nb, PB).T.copy()
    return {
        "L1": np.ascontiguousarray(L1s),
        "R1": np.ascontiguousarray(R1s),
        "biasA": np.ascontiguousarray(biasA),
    }


def build(nc, tc, ctx, aps, n=N, iters=ITERS):
    """Emit the single-core program. aps: dict name->dram AP."""
    import concourse.mybir as mybir

    f32 = mybir.dt.float32
    bf16 = mybir.dt.bfloat16
    AF = mybir.ActivationFunctionType
    ALU = mybir.AluOpType

    nb = n // PB            # 16
    nch = n // CHW          # 4
    tpc = CHW // PB         # 4
    ESCL = float(2.0 / EPS_SINKHORN)    # 200.0

    persist = ctx.enter_context(tc.tile_pool(name="persist", bufs=1))

    KA = persist.tile([PB, nb * n], bf16, tag="KA")   # [i_p, ib*n + j]
    KB = persist.tile([PB, nb * n], bf16, tag="KB")   # [j_p, jb*n + i]
    ev = persist.tile([PB, nb], bf16, tag="ev")       # e^v stationary cols
    eu = persist.tile([PB, nb], bf16, tag="eu")       # e^u stationary cols
    identB = persist.tile([PB, PB], bf16, tag="identB")
    tiny_col = persist.tile([PB, 1], f32, tag="tiny_col")
    biasA_sb = persist.tile([PB, nb], f32, tag="biasA")
    selS = persist.tile([97, tpc], bf16, tag="selS")    # selector 0/1
    ones_col = persist.tile([PB, 1], f32, tag="ones_col")

    from concourse.masks import make_identity

    nc.gpsimd.memset(tiny_col[:, :], 2e-38)
    nc.gpsimd.memset(ev[:, :], 1.0)   # e^{v_0} = 1
    nc.gpsimd.memset(selS[:, :], 0.0)
    nc.gpsimd.memset(ones_col[:, :], 1.0)
    for c in range(4):
        nc.gpsimd.memset(selS[32 * c:32 * c + 1, c:c + 1], 1.0)
    make_identity(nc, identB[:, :])
    nc.sync.dma_start(out=biasA_sb[:, :], in_=aps["biasA"][:, :])

    # ---------------- setup: K_A via matmul+exp; K_B by transposing ----------
    with tc.tile_pool(name="sin", bufs=1) as sin, \
         tc.tile_pool(name="spsum", bufs=3, space="PSUM") as sp:
        L1 = sin.tile([24, n], bf16, tag="L1")
        R1 = sin.tile([24, n], bf16, tag="R1")
        for t, name in ((L1, "L1"), (R1, "R1")):
            nc.sync.dma_start(out=t[:, :], in_=aps[name][:, :])
        pending = None
        for ib in range(nb):
            for jc in range(nch):
                P = sp.tile([PB, CHW], f32, tag="P")
                nc.tensor.matmul(
                    P[:, :],
                    lhsT=L1[:, ib * PB:(ib + 1) * PB],
                    rhs=R1[:, jc * CHW:(jc + 1) * CHW],
                    start=True, stop=True,
                )
                nc.scalar.activation(
                    KA[:, ib * n + jc * CHW: ib * n + (jc + 1) * CHW],
                    P[:, :], AF.Exp,
                    bias=biasA_sb[:, ib:ib + 1], scale=ESCL,
                )
                if pending is not None:
                    pending()
                def mk_transpose(ib=ib, jc=jc):
                    # K_B[j, i] tiles by transposing the just-built K_A chunk
                    for q in range(tpc):
                        kbt = sp.tile([PB, PB], bf16, tag="kbt", name="kbt")
                        nc.tensor.transpose(
                            kbt[:, :],
                            KA[:, ib * n + jc * CHW + q * PB:
                               ib * n + jc * CHW + (q + 1) * PB],
                            identB[:, :],
                        )
                        nc.vector.tensor_copy(
                            KB[:, (jc * tpc + q) * n + ib * PB:
                               (jc * tpc + q) * n + (ib + 1) * PB],
                            kbt[:, :],
                        )
                pending = mk_transpose
        pending()

    # ---------------- Sinkhorn iterations (4-way column-tiled) ----------------
    rp = ctx.enter_context(tc.tile_pool(name="rp", bufs=2, space="PSUM"))
    tp = ctx.enter_context(tc.tile_pool(name="tp", bufs=4, space="PSUM"))
    rows = ctx.enter_context(tc.tile_pool(name="rows", bufs=2))

    # initialize all 128 partitions of both R psum banks so the [97,512]
    # ScalarE read below never sees uninitialized psum
    for _ in range(2):
        Rinit = rp.tile([PB, CHW], f32, tag="R", name="Rinit")
        nc.tensor.matmul(Rinit[:, :], lhsT=identB[:, :], rhs=KA[:, 0:CHW],
                         start=True, stop=True)

    def half(mat, src, dst, prev_transform):
        """dst cols = C/(matvec(mat, src) + eps); returns transform closure."""
        R = rp.tile([PB, CHW], f32, tag="R", name="R")
        for g in range(nb):
            jb = PERM[g]
            if prev_transform is not None and g == 0:
                for t in range(tpc):
                    prev_transform(t)
            for q in range(4):
                nc.tensor.matmul(
                    R[32 * q:32 * q + 1, :],
                    lhsT=src[:, jb:jb + 1],
                    rhs=mat[:, jb * n + q * CHW: jb * n + (q + 1) * CHW],
                    start=(g == 0), stop=(g == nb - 1),
                    tile_position=(0, 32 * q),
                )
        srow = rows.tile([97, CHW], bf16, tag="srow", name="srow")

        def transform(t, R=R, srow=srow):
            if t == 0:
                nc.scalar.activation(
                    srow[:, :], R[0:97, :], AF.Copy,
                    bias=EPS_LOG / C_MU, scale=1.0 / C_MU)
            selps = tp.tile([PB, tpc], f32, tag="selps", name=f"selps{t}")
            nc.tensor.matmul(
                selps[:, :],
                lhsT=srow[:, PB * t:PB * (t + 1)],
                rhs=selS[:, :],
                start=True, stop=True,
            )
            # selps col c holds block (4c + t)
            dv = dst.rearrange("p (c t) -> p t c", t=tpc)[:, t, :]
            with nc.allow_low_precision(reason="ev/eu are stored bf16 anyway"):
                nc.vector.reciprocal(dv, selps[:, :])

        return transform

    # mt_jb = KB_jb * ln(KB_jb + tiny) = (K*cost)^T / -eps, built on
    # ScalarE/DVE interleaved with the iterations (they are ~85% idle).
    fin = ctx.enter_context(tc.tile_pool(name="fin", bufs=1))
    mts = []

    def build_mt(jb):
        kb_blk = KB[:, jb * n:(jb + 1) * n]
        lnk = fin.tile([PB, n], bf16, tag="lnk", bufs=2, name=f"lnk{jb}")
        nc.scalar.activation(lnk[:, :], kb_blk, AF.Ln,
                             bias=tiny_col[:, 0:1], scale=1.0)
        mt = fin.tile([PB, n], bf16, tag="mt", bufs=nb, name=f"mt{jb}")
        nc.vector.tensor_mul(mt[:, :], kb_blk, lnk[:, :])
        mts.append(mt)

    pend = None
    for it in range(iters):
        pend = half(KB, ev, eu, pend)
        if 4 <= it < 4 + nb:
            build_mt(it - 4)
        pend = half(KA, eu, ev, pend)

    # ---------------- final: emd = e^u^T (K*cost) e^v ----------------
    import os
    if os.environ.get("KCUT"):
        for t in range(tpc):
            pend(t)
        out_dbg = fin.tile([1, 1], f32, tag="out_dbg")
        nc.vector.tensor_copy(out_dbg[0:1, 0:1], ev[0:1, 0:1])
        nc.sync.dma_start(out=aps["out"][:, :], in_=out_dbg[0:1, :])
        return

    # w rows: col-tiled matvec of mt with ev stationary (consumes pend)
    W = rp.tile([PB, CHW], f32, tag="R", name="W")
    for g in range(nb):
        jb = PERM[g]
        if g == 0:
            for t in range(tpc):
                pend(t)
        for q in range(4):
            nc.tensor.matmul(
                W[32 * q:32 * q + 1, :],
                lhsT=ev[:, jb:jb + 1],
                rhs=mts[jb][:, q * CHW:(q + 1) * CHW],
                start=(g == 0), stop=(g == nb - 1),
                tile_position=(0, 32 * q),
            )

    # dot: emd = -eps * sum_i w_i * e^u_i. Transpose W's rows into columns
    # via selector matmuls, multiply by the eu columns, reduce.
    wsrow = fin.tile([97, CHW], bf16, tag="wsrow")
    nc.scalar.activation(wsrow[:, :], W[0:97, :], AF.Copy, bias=0.0, scale=1.0)
    if os.environ.get("KCUT2"):
        out_dbg = fin.tile([1, 1], f32, tag="out_dbg")
        nc.vector.tensor_copy(out_dbg[0:1, 0:1], wsrow[0:1, 0:1])
        nc.sync.dma_start(out=aps["out"][:, :], in_=out_dbg[0:1, :])
        return
    prods = fin.tile([PB, nb], f32, tag="prods")
    for t in range(tpc):
        wps = tp.tile([PB, tpc], f32, tag="selps", name=f"wps{t}")
        nc.tensor.matmul(
            wps[:, :], lhsT=wsrow[:, PB * t:PB * (t + 1)], rhs=selS[:, :],
            start=True, stop=True)
        euv = eu.rearrange("p (c t) -> p t c", t=tpc)[:, t, :]
        nc.vector.tensor_mul(prods[:, 4 * t:4 * t + 4], wps[:, :], euv)
    dots = fin.tile([PB, 1], f32, tag="dots")
    nc.vector.reduce_sum(dots[:, :], prods[:, :], axis=mybir.AxisListType.X)
    emd_ps = tp.tile([1, 1], f32, tag="selps", name="emd_ps")
    nc.tensor.matmul(emd_ps[0:1, 0:1], lhsT=dots[:, 0:1],
                     rhs=ones_col[:, 0:1], start=True, stop=True)
    out_sb = fin.tile([1, 1], f32, tag="out_sb")
    nc.scalar.activation(out_sb[0:1, :], emd_ps[0:1, :], AF.Copy,
                         bias=0.0, scale=-EPS_SINKHORN)
    nc.sync.dma_start(out=aps["out"][:, :], in_=out_sb[0:1, :])


def _build_program(n=N, iters=ITERS, debug=False):
    from contextlib import ExitStack
    import concourse.mybir as mybir
    import concourse.tile as tile
    from concourse import bacc

    f32 = mybir.dt.float32
    nb = n // PB
    nc = bacc.Bacc(
        "TRN2",
        target_bir_lowering=False,
        debug=debug,
        enable_asserts=True,
        num_devices=NCORES,
    )
    aps = {}
    for name in ("L1", "R1"):
        aps[name] = nc.dram_tensor(
            name, [24, n], mybir.dt.bfloat16, kind="ExternalInput")[:, :]
    for name in ("biasA",):
        aps[name] = nc.dram_tensor(name, [PB, nb], f32, kind="ExternalInput")[:, :]
    aps["out"] = nc.dram_tensor("out", [1, 1], f32, kind="ExternalOutput")[:, :]
    with ExitStack() as ctx:
        tc = ctx.enter_context(tile.TileContext(nc))
        build(nc, tc, ctx, aps, n=n, iters=iters)
    nc.compile()
    return nc


_CACHE = {}
LAST_RESULT = None


def _install_ntff_hook_stub():
    """concourse's trace path imports antenv.axon_hooks unconditionally;
    some images lack it.  Provide a functional stub so trace=True (e.g. a
    BASS_TRACE env in the caller) can't crash the run."""
    import sys
    import types
    try:
        import antenv.axon_hooks  # noqa: F401
        return
    except ImportError:
        pass
    hook = None
    try:
        from trn_agent_boot.trn_boot import _ntff_profile_via_ctypes
        hook = _ntff_profile_via_ctypes("/opt/axon/libaxon_pjrt.so")
    except Exception:
        hook = None
    mod = types.ModuleType("antenv.axon_hooks")
    mod.get_axon_ntff_profile_hook = lambda: hook
    mod.set_axon_ntff_profile_hook = lambda h: None
    sys.modules["antenv.axon_hooks"] = mod


def kernel(x1, x2):
    global LAST_RESULT
    _install_ntff_hook_stub()
    from concourse.bass_utils import run_bass_kernel_spmd

    x1 = np.asarray(x1, dtype=np.float32)
    x2 = np.asarray(x2, dtype=np.float32)
    B = x1.shape[0]
    assert B == NCORES and x1.shape[1] == N

    if "nc" not in _CACHE:
        _CACHE["nc"] = _build_program()
    nc = _CACHE["nc"]

    in_maps = [_host_prep(x1[b], x2[b], N) for b in range(B)]
    res = run_bass_kernel_spmd(nc, in_maps, core_ids=list(range(NCORES)))
    LAST_RESULT = res
    out = np.array([res.results[b]["out"][0, 0] for b in range(B)],
                   dtype=np.float32)
    return out


if __name__ == "__main__":
    rng = np.random.default_rng(0)
    x1 = rng.standard_normal((NCORES, N, 3)).astype(np.float32)
    x2 = rng.standard_normal((NCORES, N, 3)).astype(np.float32)
    print(kernel(x1, x2))
